# revision 1
# baseline (speedup 1.0000x reference)
"""CartesianMACE rank-0 kernel for 8 Trainium2 NeuronCores (Bass/Tile).

Only the rank-0 (scalar) channel chain affects the model output: the final
readout is sum_j h[0][:, j, 0], and h[0]'s update depends only on A[0]
(which depends only on h[0] via the scalar atomic basis) and msgs[0]
(a polynomial in A[0]).  Ranks 1/2 are dead code for this output.

Per layer l:
  hm   = h0s @ ab_w[l,0].T                                  [N,C]
  A0   = segsum(remb[e,:] * hm[src_e,:], dst)               [N,C]
  msgs = (w1 + w2*A0 + w3*A0^2) * A0 (elementwise, per ch)  [N,C]
  h0s' = einsum('njk,nk->nj', cw[l,0], h0s)
       + einsum('njk,nk->nj', mw[l,0], msgs)
out[k] = sum_n (sum_j h0s_final[n,j]) * pred_w[k,n] + pred_b[k]

Sharding: nodes are dst-sharded across the 8 cores (padded to G groups of
128 partitions per core); edges are bucketed host-side by dst into
per-node padded degree slots (uniform max degree D).  Each layer's hm
table (plus pos for layer 0's fused distance computation) is AllGathered
compactly across the cores, widened into a 256-byte-row gather table, and
edge rows are fetched with dma_gather (InstDMAGatherAnt) one group at a
time, then reduced over the degree axis on-chip.  The [N,C,C] rank-0
cw/mw weights are node-sharded and streamed.
"""

import sys

import numpy as np

if "/opt/trn_rl_repo" not in sys.path:
    sys.path.insert(0, "/opt/trn_rl_repo")

import concourse.bacc as bacc
import concourse.mybir as mybir
import concourse.tile as tile
from concourse import library_config
from concourse.bass_utils import run_bass_kernel_spmd

NCORES = 8
P = 128
ROW = 64  # gather-table row width in fp32 elems (256B, dma_gather minimum)

# debug bisection switches (leave False for real runs)
_DBG_NO_COLLECTIVE = False
_DBG_NO_GATHER = False
_DBG_NO_LIBRARY = False
_DBG_STOP = 99
_DBG_EDGE = 99  # 1=gather 2=+dist 3=+sin 4=+remb 5=+prod 6=+reduce
_DBG_SINGLE_PACKET = False
_DBG_GATHER_SPLIT = 4  # 1=setup 2=+hm0/coll0/widen 3=+edge0 4=+layer0 5=+coll1 6=full

F32 = mybir.dt.float32
I16 = mybir.dt.int16

def _build_nc(G, D, C, npad):
    """Build the SPMD Bass program.

    G: node groups of 128 per core; D: padded max in-degree; C: channels;
    npad = G*128 nodes per core.
    """
    nc = bacc.Bacc(
        "TRN2",
        target_bir_lowering=False,
        debug=False,
        num_devices=NCORES,
    )
    w0 = C + 4  # compact layer-0 row: hm | pos | pad
    ntab = NCORES * npad
    IW = 8 * D  # int16 idx columns per group

    # ---- I/O ----
    idx_in = nc.dram_tensor("idx_in", [G, P, IW], I16, kind="ExternalInput")
    msk_in = nc.dram_tensor("msk_in", [G, P, D], F32, kind="ExternalInput")
    posd_in = nc.dram_tensor("posd_in", [G, P, 3], F32, kind="ExternalInput")
    h0_in = nc.dram_tensor("h0_in", [G, P, C], F32, kind="ExternalInput")
    cw0_in = nc.dram_tensor("cw0_in", [G, P, C * C], F32, kind="ExternalInput")
    mw0_in = nc.dram_tensor("mw0_in", [G, P, C * C], F32, kind="ExternalInput")
    cw1_in = nc.dram_tensor("cw1_in", [G, P, C * C], F32, kind="ExternalInput")
    mw1_in = nc.dram_tensor("mw1_in", [G, P, C * C], F32, kind="ExternalInput")
    abw0_in = nc.dram_tensor("abw0_in", [P, C * C], F32, kind="ExternalInput")
    abw1_in = nc.dram_tensor("abw1_in", [P, C * C], F32, kind="ExternalInput")
    wsw0_in = nc.dram_tensor("wsw0_in", [P, 3 * C], F32, kind="ExternalInput")
    wsw1_in = nc.dram_tensor("wsw1_in", [P, 3 * C], F32, kind="ExternalInput")
    cvec_in = nc.dram_tensor("cvec_in", [P, C], F32, kind="ExternalInput")
    predw_in = nc.dram_tensor("predw_in", [G, P, 2], F32, kind="ExternalInput")
    predb_in = nc.dram_tensor("predb_in", [1, 2], F32, kind="ExternalInput")
    ones_in = nc.dram_tensor("ones_in", [P, 1], F32, kind="ExternalInput")
    out_t = nc.dram_tensor("out_part", [1, 2], F32, kind="ExternalOutput")

    groups = [list(range(NCORES))]

    with tile.TileContext(nc) as tc:
        with (
            tc.tile_pool(name="const", bufs=1) as cpool,
            tc.tile_pool(name="resident", bufs=1) as rpool,
            tc.tile_pool(name="gath", bufs=3) as gpool,
            tc.tile_pool(name="work", bufs=2) as wpool,
            tc.tile_pool(name="small", bufs=3) as spool,
            tc.tile_pool(name="wstream", bufs=3) as wspool,
            tc.tile_pool(name="psum", bufs=1, space="PSUM") as ppool,
            tc.tile_pool(name="dram", bufs=1, space="DRAM") as dpool,
        ):
            if not _DBG_NO_LIBRARY:
                nc.gpsimd.load_library(library_config.mlp)

            # ---- constants / resident data ----
            cvec_sb = cpool.tile([P, C], F32)
            abw0_sb = cpool.tile([P, C * C], F32)
            abw1_sb = cpool.tile([P, C * C], F32)
            wsw0_sb = cpool.tile([P, 3 * C], F32)
            wsw1_sb = cpool.tile([P, 3 * C], F32)
            ones_sb = cpool.tile([P, 1], F32)
            predb_sb = cpool.tile([1, 2], F32)
            nc.sync.dma_start(cvec_sb[:], cvec_in[:])
            nc.sync.dma_start(abw0_sb[:], abw0_in[:])
            nc.sync.dma_start(abw1_sb[:], abw1_in[:])
            nc.sync.dma_start(wsw0_sb[:], wsw0_in[:])
            nc.sync.dma_start(wsw1_sb[:], wsw1_in[:])
            nc.sync.dma_start(ones_sb[:], ones_in[:])
            nc.sync.dma_start(predb_sb[:], predb_in[:])

            idx_sb = rpool.tile([P, G * IW], I16)
            msk_sb = rpool.tile([P, G * D], F32)
            posd_sb = rpool.tile([P, G * 3], F32)
            h0_sb = rpool.tile([P, G * C], F32)
            predw_sb = rpool.tile([P, G * 2], F32)
            remb_sb = rpool.tile([P, G * D * C], F32)
            h1_sb = rpool.tile([P, G * C], F32)
            acc_sb = rpool.tile([P, 2], F32)
            nc.sync.dma_start(idx_sb[:], idx_in[:].rearrange("g p d -> p g d"))
            nc.sync.dma_start(msk_sb[:], msk_in[:].rearrange("g p d -> p g d"))
            nc.sync.dma_start(posd_sb[:], posd_in[:].rearrange("g p x -> p g x"))
            nc.sync.dma_start(h0_sb[:], h0_in[:].rearrange("g p c -> p g c"))
            nc.sync.dma_start(predw_sb[:], predw_in[:].rearrange("g p x -> p g x"))
            nc.vector.memset(acc_sb[:], 0.0)

            # ---- DRAM buffers: compact AllGather bounce + wide gather table
            ag0_in = dpool.tile([npad, w0], F32)
            ag0_out = dpool.tile([ntab, w0], F32)
            ag1_in = dpool.tile([npad, C], F32)
            ag1_out = dpool.tile([ntab, C], F32)
            wtab = dpool.tile([ntab, ROW], F32)

            def hm_group(g, h_src_ap, abw_sb, agin):
                """hm[:, j] = sum_k h[:, k] * abw[j, k]; DMA into agin rows."""
                t = wpool.tile([P, C * C], F32, name=f"hmT_{g}", tag="hmT")
                nc.vector.tensor_mul(
                    out=t[:].rearrange("p (k j) -> p k j", j=C),
                    in0=abw_sb[:].rearrange("p (k j) -> p k j", j=C),
                    in1=h_src_ap.to_broadcast([P, C, C]),
                )
                hm = spool.tile([P, C], F32, name=f"hm_{g}", tag="hm")
                nc.vector.reduce_sum(
                    out=hm[:],
                    in_=t[:].rearrange("p (k j) -> p j k", j=C),
                    axis=mybir.AxisListType.X,
                )
                dst = agin[:].rearrange("(g p) w -> p g w", p=P)[:, g, 0:C]
                nc.sync.dma_start(dst, hm[:])

            # ---- layer 0 hm (from input h0) + pos columns ----
            for g in range(G if _DBG_STOP >= 2 else 0):
                hm_group(g, h0_sb[:, g * C : (g + 1) * C], abw0_sb, ag0_in)
                pdst = ag0_in[:].rearrange("(g p) w -> p g w", p=P)[:, g, C : C + 3]
                nc.sync.dma_start(pdst, posd_sb[:, g * 3 : (g + 1) * 3])

            if _DBG_STOP >= 2:
                if _DBG_NO_COLLECTIVE:
                    nc.sync.dma_start(ag0_out[0:npad, :], ag0_in[:])
                else:
                    nc.gpsimd.collective_compute(
                        "AllGather",
                        mybir.AluOpType.bypass,
                        replica_groups=groups,
                        ins=[ag0_in[:].opt()],
                        outs=[ag0_out[:].opt()],
                    )
                # widen compact rows into the 256B-row gather table
                nc.sync.dma_start(wtab[:, 0 : C + 3], ag0_out[:, 0 : C + 3])

            def edge_layer(g, gath_w, remb_ready):
                """Gather rows, (layer 0 only) build remb, reduce to A0."""
                gt = gpool.tile([P, D * ROW], F32, name=f"gt_{g}", tag="gt")
                if _DBG_NO_GATHER:
                    nc.vector.memset(gt[:], 1.0)
                else:
                    ns = _DBG_GATHER_SPLIT
                    dq = D // ns
                    for s in range(ns):
                        nc.gpsimd.dma_gather(
                            out_ap=gt[:].rearrange("p (d e) -> p d e", e=ROW)[
                                :, s * dq : (s + 1) * dq, :
                            ],
                            in_ap=wtab[:],
                            idxs_ap=idx_sb[
                                :, g * IW + s * 8 * dq : g * IW + (s + 1) * 8 * dq
                            ],
                            num_idxs=P * dq,
                            num_idxs_reg=P * dq,
                            elem_size=ROW,
                            single_packet=_DBG_SINGLE_PACKET,
                        )
                gv = gt[:].rearrange("p (d e) -> p d e", e=ROW)
                he = gv[:, :, 0:C]
                remb_g = remb_sb[:, g * D * C : (g + 1) * D * C]
                if _DBG_EDGE < 2:
                    a0 = spool.tile([P, C], F32, name=f"a0_{g}", tag="a0")
                    nc.vector.memset(a0[:], 0.0)
                    return a0
                if not remb_ready:
                    # distances from gathered src pos vs resident dst pos
                    rel = spool.tile([P, D * 3], F32, name=f"rel_{g}", tag="rel")
                    posd_g = posd_sb[:, g * 3 : (g + 1) * 3]
                    nc.vector.tensor_sub(
                        out=rel[:].rearrange("p (d x) -> p d x", x=3),
                        in0=gv[:, :, C : C + 3],
                        in1=posd_g.to_broadcast([P, 3, D]).rearrange(
                            "p x d -> p d x"
                        ),
                    )
                    sq = spool.tile([P, D * 3], F32, name=f"sq_{g}", tag="sq")
                    nc.vector.tensor_mul(out=sq[:], in0=rel[:], in1=rel[:])
                    dd = spool.tile([P, D], F32, name=f"dd_{g}", tag="dd")
                    nc.vector.reduce_sum(
                        out=dd[:],
                        in_=sq[:].rearrange("p (d x) -> p d x", x=3),
                        axis=mybir.AxisListType.X,
                    )
                    dv = spool.tile([P, D], F32, name=f"dv_{g}", tag="dv")
                    nc.scalar.sqrt(dv[:], dd[:])
                    inv = spool.tile([P, D], F32, name=f"inv_{g}", tag="inv")
                    nc.vector.reciprocal(inv[:], dv[:])
                    wv = spool.tile([P, D], F32, name=f"wv_{g}", tag="wv")
                    nc.vector.tensor_mul(
                        out=wv[:], in0=inv[:], in1=msk_sb[:, g * D : (g + 1) * D]
                    )
                    # sinin = x2 = (c+1)*d/2 (cvec holds (c+1)/2); k = round(x2)
                    # via the fp32 magic trick; sin(pi*(c+1)*d) = sin(2pi*(x2-k))
                    if _DBG_EDGE < 3:
                        nc.vector.memset(remb_g, 0.0)
                        a0 = spool.tile([P, C], F32, name=f"a0_{g}", tag="a0")
                        nc.vector.memset(a0[:], 0.0)
                        return a0
                    sinin = wpool.tile([P, D * C], F32, name=f"sinin_{g}", tag="sin")
                    nc.vector.tensor_mul(
                        out=sinin[:].rearrange("p (d c) -> p d c", c=C),
                        in0=dv[:].to_broadcast([P, D, C]),
                        in1=cvec_sb[:].to_broadcast([P, C, D]).rearrange(
                            "p c d -> p d c"
                        ),
                    )
                    MAGIC = 12582912.0  # 1.5 * 2**23
                    kr = wpool.tile([P, D * C], F32, name=f"kr_{g}", tag="kr")
                    nc.vector.tensor_scalar(
                        out=kr[:],
                        in0=sinin[:],
                        scalar1=MAGIC,
                        scalar2=MAGIC,
                        op0=mybir.AluOpType.add,
                        op1=mybir.AluOpType.subtract,
                    )
                    nc.vector.tensor_sub(out=sinin[:], in0=sinin[:], in1=kr[:])
                    sino = wpool.tile([P, D * C], F32, name=f"sino_{g}", tag="sino")
                    nc.scalar.activation(
                        sino[:],
                        sinin[:],
                        mybir.ActivationFunctionType.Sin,
                        scale=float(2.0 * np.pi),
                    )
                    if _DBG_EDGE < 4:
                        nc.vector.memset(remb_g, 0.0)
                        a0 = spool.tile([P, C], F32, name=f"a0_{g}", tag="a0")
                        nc.vector.memset(a0[:], 0.0)
                        return a0
                    nc.vector.tensor_mul(
                        out=remb_g.rearrange("p (d c) -> p d c", c=C),
                        in0=sino[:].rearrange("p (d c) -> p d c", c=C),
                        in1=wv[:].to_broadcast([P, D, C]),
                    )
                if _DBG_EDGE < 5:
                    a0 = spool.tile([P, C], F32, name=f"a0_{g}", tag="a0")
                    nc.vector.memset(a0[:], 0.0)
                    return a0
                # edge products, in place over the gathered hm columns
                nc.vector.tensor_mul(
                    out=he,
                    in0=remb_g.rearrange("p (d c) -> p d c", c=C),
                    in1=he,
                )
                a0 = spool.tile([P, C], F32, name=f"a0_{g}", tag="a0")
                if _DBG_EDGE < 6:
                    nc.vector.memset(a0[:], 0.0)
                    return a0
                nc.vector.reduce_sum(
                    out=a0[:],
                    in_=gt[:].rearrange("p (d e) -> p e d", e=ROW)[:, 0:C, :],
                    axis=mybir.AxisListType.X,
                )
                return a0

            def node_update(g, a0, wsw_sb, cw_in_t, mw_in_t, h_src_ap, h_dst_ap):
                """msgs = (w1 + w2*s + w3*s^2)*A0; h' = cw@h + mw@msgs."""
                s2 = spool.tile([P, C], F32, name=f"s2_{g}", tag="s2")
                nc.scalar.square(s2[:], a0[:])
                s3 = spool.tile([P, C], F32, name=f"s3_{g}", tag="s3")
                nc.vector.tensor_mul(out=s3[:], in0=s2[:], in1=a0[:])
                m1 = spool.tile([P, C], F32, name=f"m1_{g}", tag="m1")
                nc.vector.tensor_mul(out=m1[:], in0=a0[:], in1=wsw_sb[:, 0:C])
                m2 = spool.tile([P, C], F32, name=f"m2_{g}", tag="m2")
                nc.vector.tensor_mul(out=m2[:], in0=s2[:], in1=wsw_sb[:, C : 2 * C])
                m3 = spool.tile([P, C], F32, name=f"m3_{g}", tag="m3")
                nc.vector.tensor_mul(
                    out=m3[:], in0=s3[:], in1=wsw_sb[:, 2 * C : 3 * C]
                )
                ms = spool.tile([P, C], F32, name=f"ms_{g}", tag="ms")
                nc.vector.tensor_add(out=ms[:], in0=m1[:], in1=m2[:])
                msgs = spool.tile([P, C], F32, name=f"msgs_{g}", tag="msgs")
                nc.vector.tensor_add(out=msgs[:], in0=ms[:], in1=m3[:])

                cw_t = wspool.tile([P, C * C], F32, name=f"cw_{g}", tag="cw")
                mw_t = wspool.tile([P, C * C], F32, name=f"mw_{g}", tag="mw")
                nc.sync.dma_start(cw_t[:], cw_in_t[g])
                nc.sync.dma_start(mw_t[:], mw_in_t[g])
                t1 = wpool.tile([P, C * C], F32, name=f"t1_{g}", tag="t1")
                nc.vector.tensor_mul(
                    out=t1[:].rearrange("p (k j) -> p k j", j=C),
                    in0=cw_t[:].rearrange("p (k j) -> p k j", j=C),
                    in1=h_src_ap.to_broadcast([P, C, C]),
                )
                t2 = wpool.tile([P, C * C], F32, name=f"t2_{g}", tag="t2")
                nc.vector.tensor_mul(
                    out=t2[:].rearrange("p (k j) -> p k j", j=C),
                    in0=mw_t[:].rearrange("p (k j) -> p k j", j=C),
                    in1=msgs[:].to_broadcast([P, C, C]),
                )
                ts = wpool.tile([P, C * C], F32, name=f"ts_{g}", tag="ts")
                nc.vector.tensor_add(out=ts[:], in0=t1[:], in1=t2[:])
                nc.vector.reduce_sum(
                    out=h_dst_ap,
                    in_=ts[:].rearrange("p (k j) -> p j k", j=C),
                    axis=mybir.AxisListType.X,
                )

            # ---- layer 0 ----
            for g in range(G if _DBG_STOP >= 3 else 0):
                a0 = edge_layer(g, C + 3, remb_ready=False)
                if _DBG_STOP < 4:
                    continue
                h1_g = h1_sb[:, g * C : (g + 1) * C]
                node_update(
                    g, a0, wsw0_sb, cw0_in, mw0_in,
                    h0_sb[:, g * C : (g + 1) * C], h1_g,
                )
                hm_group(g, h1_g, abw1_sb, ag1_in)

            if _DBG_STOP >= 5:
                if _DBG_NO_COLLECTIVE:
                    nc.sync.dma_start(ag1_out[0:npad, :], ag1_in[:])
                else:
                    nc.gpsimd.collective_compute(
                        "AllGather",
                        mybir.AluOpType.bypass,
                        replica_groups=groups,
                        ins=[ag1_in[:].opt()],
                        outs=[ag1_out[:].opt()],
                    )
                nc.sync.dma_start(wtab[:, 0:C], ag1_out[:, 0:C])

            # ---- layer 1 + head ----
            for g in range(G if _DBG_STOP >= 6 else 0):
                a1 = edge_layer(g, C, remb_ready=True)
                hf = spool.tile([P, C], F32, name=f"hf_{g}", tag="hf")
                node_update(
                    g, a1, wsw1_sb, cw1_in, mw1_in,
                    h1_sb[:, g * C : (g + 1) * C], hf[:],
                )
                rs = spool.tile([P, 1], F32, name=f"rs_{g}", tag="rs")
                nc.vector.reduce_sum(out=rs[:], in_=hf[:], axis=mybir.AxisListType.X)
                ctr = spool.tile([P, 2], F32, name=f"ctr_{g}", tag="ctr")
                nc.vector.tensor_mul(
                    out=ctr[:],
                    in0=predw_sb[:, g * 2 : (g + 1) * 2],
                    in1=rs[:].to_broadcast([P, 2]),
                )
                nc.vector.tensor_add(out=acc_sb[:], in0=acc_sb[:], in1=ctr[:])

            # cross-partition sum via PE, add bias, write out
            fin_ps = ppool.tile([1, 2], F32)
            nc.tensor.matmul(
                out=fin_ps[:], lhsT=ones_sb[:], rhs=acc_sb[:], start=True, stop=True
            )
            res = spool.tile([1, 2], F32)
            nc.vector.tensor_add(out=res[:], in0=fin_ps[:], in1=predb_sb[:])
            nc.sync.dma_start(out_t[:], res[:])

    nc.compile()
    return nc


def _wrap_idx16(v):
    """[128, D] int array -> dma_gather idx tile [128, 8*D] int16.

    Gathered index j = d*128 + p must sit at [j % 16, j // 16] in a
    16-partition wrap, replicated 8x down the partitions."""
    p128, d_pad = v.shape
    blk = v.T.reshape(d_pad, 8, 16).transpose(2, 0, 1).reshape(16, 8 * d_pad)
    return np.tile(blk, (8, 1)).astype(np.int16)


def _prep_inputs(pos, h0, ab_w, ws_w, cw, mw, pred_w, pred_b, edge_index):
    """Host-side sharding: bucket edges by dst shard, build padded degree
    slot tables, slice per-node weights.  Data movement / index arithmetic
    only — all model arithmetic runs on device."""
    pos = np.asarray(pos, np.float32)
    h0 = np.asarray(h0, np.float32)
    ab_w = np.asarray(ab_w, np.float32)
    ws_w = np.asarray(ws_w, np.float32)
    cw = np.asarray(cw, np.float32)
    mw = np.asarray(mw, np.float32)
    pred_w = np.asarray(pred_w, np.float32)
    pred_b = np.asarray(pred_b, np.float32)
    ei = np.asarray(edge_index)
    src = ei[0].astype(np.int64)
    dst = ei[1].astype(np.int64)

    n_nodes = pos.shape[0]
    c_ch = h0.shape[1]
    assert n_nodes % NCORES == 0
    nc_nodes = n_nodes // NCORES  # real nodes per core
    g_groups = -(-nc_nodes // P)
    npad = g_groups * P
    assert NCORES * npad <= 32767, "int16 gather indices"

    deg = np.bincount(dst, minlength=n_nodes)
    d_max = int(deg.max())
    d_pad = max(4, -(-d_max // 2) * 2)

    def padded_row(node):
        return (node // nc_nodes) * npad + (node % nc_nodes)

    in_maps = []
    for core in range(NCORES):
        lo, hi = core * nc_nodes, (core + 1) * nc_nodes
        sel = (dst >= lo) & (dst < hi)
        d_loc = (dst[sel] - lo).astype(np.int64)
        s_glb = src[sel]
        order = np.argsort(d_loc, kind="stable")
        d_sort = d_loc[order]
        s_sort = s_glb[order]
        starts = np.searchsorted(d_sort, np.arange(nc_nodes))
        rank = np.arange(len(d_sort)) - starts[d_sort]

        dummy = padded_row(hi % n_nodes)  # a node another core owns
        idx = np.full((npad, d_pad), dummy, np.int64)
        msk = np.zeros((npad, d_pad), np.float32)
        idx[d_sort, rank] = padded_row(s_sort)
        msk[d_sort, rank] = np.sqrt(2.0, dtype=np.float32)

        # wrap per group into the dma_gather int16 layout
        idx_w = np.stack(
            [
                _wrap_idx16(idx[g * P : (g + 1) * P])
                for g in range(g_groups)
            ]
        )

        posd = np.zeros((npad, 3), np.float32)
        posd[:nc_nodes] = pos[lo:hi]
        # pad owners: any pos distinct from every gathered row
        posd[nc_nodes:, 0] = 1e4 + np.arange(npad - nc_nodes, dtype=np.float32)

        h0loc = np.zeros((npad, c_ch), np.float32)
        h0loc[:nc_nodes] = h0[lo:hi, :, 0]

        def node_w(warr, layer):
            wloc = np.zeros((npad, c_ch * c_ch), np.float32)
            # pack transposed: flat (k, j) with w[j, k] at k*C + j
            wloc[:nc_nodes] = (
                warr[layer, 0, lo:hi].transpose(0, 2, 1).reshape(nc_nodes, -1)
            )
            return wloc.reshape(g_groups, P, c_ch * c_ch)

        predw = np.zeros((npad, 2), np.float32)
        predw[:nc_nodes] = pred_w[:, lo:hi].T

        rep = lambda v: np.broadcast_to(
            np.asarray(v, np.float32).reshape(1, -1), (P, np.asarray(v).size)
        ).copy()

        in_maps.append(
            {
                "idx_in": idx_w,
                "msk_in": msk.reshape(g_groups, P, d_pad),
                "posd_in": posd.reshape(g_groups, P, 3),
                "h0_in": h0loc.reshape(g_groups, P, c_ch),
                "cw0_in": node_w(cw, 0),
                "mw0_in": node_w(mw, 0),
                "cw1_in": node_w(cw, 1),
                "mw1_in": node_w(mw, 1),
                "abw0_in": rep(ab_w[0, 0].T.ravel()),
                "abw1_in": rep(ab_w[1, 0].T.ravel()),
                "wsw0_in": rep(ws_w[0, 0].ravel()),
                "wsw1_in": rep(ws_w[1, 0].ravel()),
                "cvec_in": rep(np.arange(1, c_ch + 1, dtype=np.float32) / 2.0),
                "predw_in": predw.reshape(g_groups, P, 2),
                "predb_in": (pred_b if core == 0 else np.zeros(2)).reshape(1, 2)
                .astype(np.float32),
                "ones_in": np.ones((P, 1), np.float32),
            }
        )
    meta = dict(G=g_groups, D=d_pad, C=c_ch, npad=npad)
    return in_maps, meta


_NC_CACHE = {}


def kernel(**inputs) -> np.ndarray:
    in_maps, meta = _prep_inputs(**inputs)
    key = tuple(sorted(meta.items()))
    if key not in _NC_CACHE:
        _NC_CACHE[key] = _build_nc(**meta)
    nc = _NC_CACHE[key]
    res = run_bass_kernel_spmd(nc, in_maps, core_ids=list(range(NCORES)))
    parts = [r["out_part"] for r in res.results]
    return np.sum(parts, axis=0).astype(np.float32)



# revision 7
# speedup vs baseline: 3.7628x; 3.7628x over previous
"""CartesianMACE rank-0 kernel for 8 Trainium2 NeuronCores (Bass/Tile).

Only the rank-0 (scalar) channel chain affects the model output: the final
readout is sum_j h[0][:, j, 0], and h[0]'s update depends only on A[0]
(which depends only on h[0] via the scalar atomic basis) and msgs[0]
(a polynomial in A[0]).  Ranks 1/2 are dead code for this output.

Per layer l:
  hm   = h0s @ ab_w[l,0].T                                  [N,C]
  A0   = segsum(remb[e,:] * hm[src_e,:], dst)               [N,C]
  msgs = (w1 + w2*A0 + w3*A0^2) * A0 (elementwise, per ch)  [N,C]
  h0s' = einsum('njk,nk->nj', cw[l,0], h0s)
       + einsum('njk,nk->nj', mw[l,0], msgs)
out[k] = sum_n (sum_j h0s_final[n,j]) * pred_w[k,n] + pred_b[k]

Sharding: nodes are dst-sharded across the 8 cores; within a core, nodes
are sorted by in-degree (descending) and grouped into G groups of 128
partitions with a per-group padded degree D_g (multiple of 4) — this cuts
padded edge slots ~1.7x vs a global max degree.  Edges are bucketed
host-side by dst into per-node degree slots.  Each layer's hm table (plus
pos for layer 0's fused distance computation) is AllGathered compactly
across the cores, widened into a 256-byte-row gather table, and edge rows
are fetched with dma_gather (InstDMAGatherAnt).  dma_gather descriptor
generation on the Pool engine runs on one Q7 core-pair per SWDGE queue, so
calls rotate across 4 queues (num_swdge_queues=4) for 4x desc-gen
throughput.  The [N,C,C] rank-0 cw/mw weights are node-sharded/streamed.
"""

import sys

import numpy as np

if "/opt/trn_rl_repo" not in sys.path:
    sys.path.insert(0, "/opt/trn_rl_repo")

import concourse.bacc as bacc
import concourse.mybir as mybir
import concourse.tile as tile
from concourse import library_config
from concourse.bass_utils import run_bass_kernel_spmd

NCORES = 8
P = 128
ROW = 64  # gather-table row width in fp32 elems (256B, dma_gather minimum)
NSPLIT = 4  # gather splits per group == SWDGE queue rotation width

F32 = mybir.dt.float32
I16 = mybir.dt.int16


def _build_nc(Dg, C, npad):
    """Build the SPMD Bass program.

    Dg: tuple of per-group padded max in-degrees (len G, each % 4 == 0);
    C: channels; npad: padded nodes per core (G*128).
    """
    Dg = list(Dg)
    G = len(Dg)
    nc = bacc.Bacc(
        "TRN2",
        target_bir_lowering=False,
        debug=False,
        num_devices=NCORES,
        num_swdge_queues=NSPLIT,
    )
    w0 = C + 4  # compact layer-0 row: hm | pos | pad
    ntab = NCORES * npad
    IW = sum(8 * d for d in Dg)   # int16 idx columns total
    MW = sum(Dg)                  # mask columns total
    RW = sum(d * C for d in Dg)   # remb columns total
    # per-group offsets
    off_i = np.cumsum([0] + [8 * d for d in Dg]).tolist()
    off_m = np.cumsum([0] + list(Dg)).tolist()
    off_r = np.cumsum([0] + [d * C for d in Dg]).tolist()

    qctr = [0]

    def next_q():
        q = qctr[0] % NSPLIT
        qctr[0] += 1
        return q

    # ---- I/O ----
    idx_in = nc.dram_tensor("idx_in", [P, IW], I16, kind="ExternalInput")
    msk_in = nc.dram_tensor("msk_in", [P, MW], F32, kind="ExternalInput")
    posd_in = nc.dram_tensor("posd_in", [P, G * 3], F32, kind="ExternalInput")
    h0_in = nc.dram_tensor("h0_in", [P, G * C], F32, kind="ExternalInput")
    cw0_in = nc.dram_tensor("cw0_in", [G, P, C * C], F32, kind="ExternalInput")
    mw0_in = nc.dram_tensor("mw0_in", [G, P, C * C], F32, kind="ExternalInput")
    cw1_in = nc.dram_tensor("cw1_in", [G, P, C * C], F32, kind="ExternalInput")
    mw1_in = nc.dram_tensor("mw1_in", [G, P, C * C], F32, kind="ExternalInput")
    abw0_in = nc.dram_tensor("abw0_in", [P, C * C], F32, kind="ExternalInput")
    abw1_in = nc.dram_tensor("abw1_in", [P, C * C], F32, kind="ExternalInput")
    wsw0_in = nc.dram_tensor("wsw0_in", [P, 3 * C], F32, kind="ExternalInput")
    wsw1_in = nc.dram_tensor("wsw1_in", [P, 3 * C], F32, kind="ExternalInput")
    cvec_in = nc.dram_tensor("cvec_in", [P, C], F32, kind="ExternalInput")
    predw_in = nc.dram_tensor("predw_in", [P, G * 2], F32, kind="ExternalInput")
    predb_in = nc.dram_tensor("predb_in", [1, 2], F32, kind="ExternalInput")
    ones_in = nc.dram_tensor("ones_in", [P, 1], F32, kind="ExternalInput")
    out_t = nc.dram_tensor("out_part", [1, 2], F32, kind="ExternalOutput")

    groups = [list(range(NCORES))]

    with tile.TileContext(nc) as tc:
        with (
            tc.tile_pool(name="const", bufs=1) as cpool,
            tc.tile_pool(name="resident", bufs=1) as rpool,
            tc.tile_pool(name="gath", bufs=3) as gpool,
            tc.tile_pool(name="work", bufs=2) as wpool,
            tc.tile_pool(name="small", bufs=3) as spool,
            tc.tile_pool(name="wstream", bufs=3) as wspool,
            tc.tile_pool(name="psum", bufs=1, space="PSUM") as ppool,
            tc.tile_pool(name="dram", bufs=1, space="DRAM") as dpool,
        ):
            nc.gpsimd.load_library(library_config.mlp)

            # ---- constants / resident data ----
            cvec_sb = cpool.tile([P, C], F32)
            abw0_sb = cpool.tile([P, C * C], F32)
            abw1_sb = cpool.tile([P, C * C], F32)
            wsw0_sb = cpool.tile([P, 3 * C], F32)
            wsw1_sb = cpool.tile([P, 3 * C], F32)
            ones_sb = cpool.tile([P, 1], F32)
            predb_sb = cpool.tile([1, 2], F32)
            nc.sync.dma_start(cvec_sb[:], cvec_in[:])
            nc.sync.dma_start(abw0_sb[:], abw0_in[:])
            nc.sync.dma_start(abw1_sb[:], abw1_in[:])
            nc.sync.dma_start(wsw0_sb[:], wsw0_in[:])
            nc.sync.dma_start(wsw1_sb[:], wsw1_in[:])
            nc.sync.dma_start(ones_sb[:], ones_in[:])
            nc.sync.dma_start(predb_sb[:], predb_in[:])

            idx_sb = rpool.tile([P, IW], I16)
            msk_sb = rpool.tile([P, MW], F32)
            posd_sb = rpool.tile([P, G * 3], F32)
            h0_sb = rpool.tile([P, G * C], F32)
            predw_sb = rpool.tile([P, G * 2], F32)
            remb_sb = rpool.tile([P, RW], F32)
            h1_sb = rpool.tile([P, G * C], F32)
            acc_sb = rpool.tile([P, 2], F32)
            nc.sync.dma_start(idx_sb[:], idx_in[:])
            nc.sync.dma_start(msk_sb[:], msk_in[:])
            nc.sync.dma_start(posd_sb[:], posd_in[:])
            nc.sync.dma_start(h0_sb[:], h0_in[:])
            nc.sync.dma_start(predw_sb[:], predw_in[:])
            nc.vector.memset(acc_sb[:], 0.0)

            # ---- DRAM buffers: compact AllGather bounce + wide gather table
            ag0_in = dpool.tile([npad, w0], F32)
            ag0_out = dpool.tile([ntab, w0], F32)
            ag1_in = dpool.tile([npad, C], F32)
            ag1_out = dpool.tile([ntab, C], F32)
            wtab = dpool.tile([ntab, ROW], F32)

            def hm_group(g, h_src_ap, abw_sb, agin):
                """hm[:, j] = sum_k h[:, k] * abw[j, k]; DMA into agin rows."""
                t = wpool.tile([P, C * C], F32, name=f"hmT_{g}", tag="hmT")
                nc.vector.tensor_mul(
                    out=t[:].rearrange("p (k j) -> p k j", j=C),
                    in0=abw_sb[:].rearrange("p (k j) -> p k j", j=C),
                    in1=h_src_ap.to_broadcast([P, C, C]),
                )
                hm = spool.tile([P, C], F32, name=f"hm_{g}", tag="hm")
                nc.vector.reduce_sum(
                    out=hm[:],
                    in_=t[:].rearrange("p (k j) -> p j k", j=C),
                    axis=mybir.AxisListType.X,
                )
                dst = agin[:].rearrange("(g p) w -> p g w", p=P)[:, g, 0:C]
                nc.sync.dma_start(dst, hm[:])

            # ---- layer 0 hm (from input h0) + pos columns ----
            for g in range(G):
                hm_group(g, h0_sb[:, g * C : (g + 1) * C], abw0_sb, ag0_in)
                pdst = ag0_in[:].rearrange("(g p) w -> p g w", p=P)[:, g, C : C + 3]
                nc.sync.dma_start(pdst, posd_sb[:, g * 3 : (g + 1) * 3])

            nc.gpsimd.collective_compute(
                "AllGather",
                mybir.AluOpType.bypass,
                replica_groups=groups,
                ins=[ag0_in[:].opt()],
                outs=[ag0_out[:].opt()],
            )
            # widen compact rows into the 256B-row gather table
            nc.sync.dma_start(wtab[:, 0 : C + 3], ag0_out[:, 0 : C + 3])

            Dmax = max(Dg)

            def edge_layer(g, remb_ready):
                """Gather rows, (layer 0 only) build remb, reduce to A0."""
                D = Dg[g]
                gt = gpool.tile([P, Dmax * ROW], F32, name=f"gt_{g}", tag="gt")
                dq = D // NSPLIT
                for s in range(NSPLIT):
                    nc.gpsimd.dma_gather(
                        out_ap=gt[:].rearrange("p (d e) -> p d e", e=ROW)[
                            :, s * dq : (s + 1) * dq, :
                        ],
                        in_ap=wtab[:],
                        idxs_ap=idx_sb[
                            :,
                            off_i[g] + s * 8 * dq : off_i[g] + (s + 1) * 8 * dq,
                        ],
                        num_idxs=P * dq,
                        num_idxs_reg=P * dq,
                        elem_size=ROW,
                        single_packet=False,
                        queue_num=next_q(),
                    )
                gv = gt[:].rearrange("p (d e) -> p d e", e=ROW)[:, 0:D, :]
                he = gt[:].rearrange("p (d e) -> p d e", e=ROW)[:, 0:D, 0:C]
                remb_g = remb_sb[:, off_r[g] : off_r[g] + D * C]
                if not remb_ready:
                    # distances from gathered src pos vs resident dst pos
                    rel_t = spool.tile([P, Dmax * 3], F32, name=f"rel_{g}", tag="rel")
                    rel = rel_t[:, 0 : D * 3]
                    posd_g = posd_sb[:, g * 3 : (g + 1) * 3]
                    nc.vector.tensor_sub(
                        out=rel.rearrange("p (d x) -> p d x", x=3),
                        in0=gv[:, :, C : C + 3],
                        in1=posd_g.to_broadcast([P, 3, D]).rearrange(
                            "p x d -> p d x"
                        ),
                    )
                    sq_t = spool.tile([P, Dmax * 3], F32, name=f"sq_{g}", tag="sq")
                    sq = sq_t[:, 0 : D * 3]
                    nc.vector.tensor_mul(out=sq, in0=rel, in1=rel)
                    dd_t = spool.tile([P, Dmax], F32, name=f"dd_{g}", tag="dd")
                    dd = dd_t[:, 0:D]
                    nc.vector.reduce_sum(
                        out=dd,
                        in_=sq.rearrange("p (d x) -> p d x", x=3),
                        axis=mybir.AxisListType.X,
                    )
                    dv_t = spool.tile([P, Dmax], F32, name=f"dv_{g}", tag="dv")
                    dv = dv_t[:, 0:D]
                    nc.scalar.sqrt(dv, dd)
                    inv_t = spool.tile([P, Dmax], F32, name=f"inv_{g}", tag="inv")
                    inv = inv_t[:, 0:D]
                    nc.vector.reciprocal(inv, dv)
                    wv_t = spool.tile([P, Dmax], F32, name=f"wv_{g}", tag="wv")
                    wv = wv_t[:, 0:D]
                    nc.vector.tensor_mul(
                        out=wv, in0=inv, in1=msk_sb[:, off_m[g] : off_m[g] + D]
                    )
                    # sinin = x2 = (c+1)*d/2 (cvec holds (c+1)/2); k = round(x2)
                    # via the fp32 magic trick; sin(pi*(c+1)*d) = sin(2pi*(x2-k))
                    sin_t = wpool.tile([P, Dmax * C], F32, name=f"sinin_{g}", tag="sin")
                    sinin = sin_t[:, 0 : D * C]
                    nc.vector.tensor_mul(
                        out=sinin.rearrange("p (d c) -> p d c", c=C),
                        in0=dv.to_broadcast([P, D, C]),
                        in1=cvec_sb[:].to_broadcast([P, C, D]).rearrange(
                            "p c d -> p d c"
                        ),
                    )
                    MAGIC = 12582912.0  # 1.5 * 2**23
                    kr_t = wpool.tile([P, Dmax * C], F32, name=f"kr_{g}", tag="kr")
                    kr = kr_t[:, 0 : D * C]
                    nc.vector.tensor_scalar(
                        out=kr,
                        in0=sinin,
                        scalar1=MAGIC,
                        scalar2=MAGIC,
                        op0=mybir.AluOpType.add,
                        op1=mybir.AluOpType.subtract,
                    )
                    nc.vector.tensor_sub(out=sinin, in0=sinin, in1=kr)
                    sino_t = wpool.tile([P, Dmax * C], F32, name=f"sino_{g}", tag="sino")
                    sino = sino_t[:, 0 : D * C]
                    nc.scalar.activation(
                        sino,
                        sinin,
                        mybir.ActivationFunctionType.Sin,
                        scale=float(2.0 * np.pi),
                    )
                    nc.vector.tensor_mul(
                        out=remb_g.rearrange("p (d c) -> p d c", c=C),
                        in0=sino.rearrange("p (d c) -> p d c", c=C),
                        in1=wv.to_broadcast([P, D, C]),
                    )
                # edge products, in place over the gathered hm columns
                nc.vector.tensor_mul(
                    out=he,
                    in0=remb_g.rearrange("p (d c) -> p d c", c=C),
                    in1=he,
                )
                a0 = spool.tile([P, C], F32, name=f"a0_{g}", tag="a0")
                nc.vector.reduce_sum(
                    out=a0[:],
                    in_=gt[:].rearrange("p (d e) -> p e d", e=ROW)[:, 0:C, 0:D],
                    axis=mybir.AxisListType.X,
                )
                return a0

            def node_update(g, a0, wsw_sb, cw_in_t, mw_in_t, h_src_ap, h_dst_ap):
                """msgs = (w1 + w2*s + w3*s^2)*A0; h' = cw@h + mw@msgs."""
                s2 = spool.tile([P, C], F32, name=f"s2_{g}", tag="s2")
                nc.scalar.square(s2[:], a0[:])
                s3 = spool.tile([P, C], F32, name=f"s3_{g}", tag="s3")
                nc.vector.tensor_mul(out=s3[:], in0=s2[:], in1=a0[:])
                m1 = spool.tile([P, C], F32, name=f"m1_{g}", tag="m1")
                nc.vector.tensor_mul(out=m1[:], in0=a0[:], in1=wsw_sb[:, 0:C])
                m2 = spool.tile([P, C], F32, name=f"m2_{g}", tag="m2")
                nc.vector.tensor_mul(out=m2[:], in0=s2[:], in1=wsw_sb[:, C : 2 * C])
                m3 = spool.tile([P, C], F32, name=f"m3_{g}", tag="m3")
                nc.vector.tensor_mul(
                    out=m3[:], in0=s3[:], in1=wsw_sb[:, 2 * C : 3 * C]
                )
                ms = spool.tile([P, C], F32, name=f"ms_{g}", tag="ms")
                nc.vector.tensor_add(out=ms[:], in0=m1[:], in1=m2[:])
                msgs = spool.tile([P, C], F32, name=f"msgs_{g}", tag="msgs")
                nc.vector.tensor_add(out=msgs[:], in0=ms[:], in1=m3[:])

                cw_t = wspool.tile([P, C * C], F32, name=f"cw_{g}", tag="cw")
                mw_t = wspool.tile([P, C * C], F32, name=f"mw_{g}", tag="mw")
                nc.sync.dma_start(cw_t[:], cw_in_t[g])
                nc.sync.dma_start(mw_t[:], mw_in_t[g])
                t1 = wpool.tile([P, C * C], F32, name=f"t1_{g}", tag="t1")
                nc.vector.tensor_mul(
                    out=t1[:].rearrange("p (k j) -> p k j", j=C),
                    in0=cw_t[:].rearrange("p (k j) -> p k j", j=C),
                    in1=h_src_ap.to_broadcast([P, C, C]),
                )
                t2 = wpool.tile([P, C * C], F32, name=f"t2_{g}", tag="t2")
                nc.vector.tensor_mul(
                    out=t2[:].rearrange("p (k j) -> p k j", j=C),
                    in0=mw_t[:].rearrange("p (k j) -> p k j", j=C),
                    in1=msgs[:].to_broadcast([P, C, C]),
                )
                ts = wpool.tile([P, C * C], F32, name=f"ts_{g}", tag="ts")
                nc.vector.tensor_add(out=ts[:], in0=t1[:], in1=t2[:])
                nc.vector.reduce_sum(
                    out=h_dst_ap,
                    in_=ts[:].rearrange("p (k j) -> p j k", j=C),
                    axis=mybir.AxisListType.X,
                )

            # ---- layer 0 ----
            for g in range(G):
                a0 = edge_layer(g, remb_ready=False)
                h1_g = h1_sb[:, g * C : (g + 1) * C]
                node_update(
                    g, a0, wsw0_sb, cw0_in, mw0_in,
                    h0_sb[:, g * C : (g + 1) * C], h1_g,
                )
                hm_group(g, h1_g, abw1_sb, ag1_in)

            nc.gpsimd.collective_compute(
                "AllGather",
                mybir.AluOpType.bypass,
                replica_groups=groups,
                ins=[ag1_in[:].opt()],
                outs=[ag1_out[:].opt()],
            )
            nc.sync.dma_start(wtab[:, 0:C], ag1_out[:, 0:C])

            # ---- layer 1 + head ----
            for g in range(G):
                a1 = edge_layer(g, remb_ready=True)
                hf = spool.tile([P, C], F32, name=f"hf_{g}", tag="hf")
                node_update(
                    g, a1, wsw1_sb, cw1_in, mw1_in,
                    h1_sb[:, g * C : (g + 1) * C], hf[:],
                )
                rs = spool.tile([P, 1], F32, name=f"rs_{g}", tag="rs")
                nc.vector.reduce_sum(out=rs[:], in_=hf[:], axis=mybir.AxisListType.X)
                ctr = spool.tile([P, 2], F32, name=f"ctr_{g}", tag="ctr")
                nc.vector.tensor_mul(
                    out=ctr[:],
                    in0=predw_sb[:, g * 2 : (g + 1) * 2],
                    in1=rs[:].to_broadcast([P, 2]),
                )
                nc.vector.tensor_add(out=acc_sb[:], in0=acc_sb[:], in1=ctr[:])

            # cross-partition sum via PE, add bias, write out
            fin_ps = ppool.tile([1, 2], F32)
            nc.tensor.matmul(
                out=fin_ps[:], lhsT=ones_sb[:], rhs=acc_sb[:], start=True, stop=True
            )
            res = spool.tile([1, 2], F32)
            nc.vector.tensor_add(out=res[:], in0=fin_ps[:], in1=predb_sb[:])
            nc.sync.dma_start(out_t[:], res[:])

    nc.compile()
    return nc


def _wrap_idx16(v):
    """[128, D] int array -> dma_gather idx tile [128, 8*D] int16.

    Gathered index j = d*128 + p must sit at [j % 16, j // 16] in a
    16-partition wrap, replicated 8x down the partitions."""
    p128, d_pad = v.shape
    blk = v.T.reshape(d_pad, 8, 16).transpose(2, 0, 1).reshape(16, 8 * d_pad)
    return np.tile(blk, (8, 1)).astype(np.int16)


def _prep_inputs(pos, h0, ab_w, ws_w, cw, mw, pred_w, pred_b, edge_index):
    """Host-side sharding: degree-sort nodes per core, bucket edges by dst
    into per-group padded degree slots, slice per-node weights.  Data
    movement / index arithmetic only — all model arithmetic runs on
    device."""
    pos = np.asarray(pos, np.float32)
    h0 = np.asarray(h0, np.float32)
    ab_w = np.asarray(ab_w, np.float32)
    ws_w = np.asarray(ws_w, np.float32)
    cw = np.asarray(cw, np.float32)
    mw = np.asarray(mw, np.float32)
    pred_w = np.asarray(pred_w, np.float32)
    pred_b = np.asarray(pred_b, np.float32)
    ei = np.asarray(edge_index)
    src = ei[0].astype(np.int64)
    dst = ei[1].astype(np.int64)

    n_nodes = pos.shape[0]
    c_ch = h0.shape[1]
    assert n_nodes % NCORES == 0
    nc_nodes = n_nodes // NCORES  # real nodes per core
    g_groups = -(-nc_nodes // P)
    npad = g_groups * P
    assert NCORES * npad <= 32767, "int16 gather indices"

    deg_all = np.bincount(dst, minlength=n_nodes)

    # per-core degree-descending node permutation (padding nodes at end)
    perms = []     # perms[core][slot] = local real node id (or >=nc_nodes pad)
    inv_slot = np.zeros(n_nodes, np.int64)  # global node -> slot within core
    for core in range(NCORES):
        lo = core * nc_nodes
        order = np.argsort(-deg_all[lo : lo + nc_nodes], kind="stable")
        perm = np.concatenate([order, np.arange(nc_nodes, npad)])
        perms.append(perm)
        inv_slot[lo + order] = np.arange(nc_nodes)

    # per-group padded degree (shared across cores so one program serves all)
    deg_sorted = np.zeros((NCORES, npad), np.int64)
    for core in range(NCORES):
        lo = core * nc_nodes
        deg_sorted[core, :nc_nodes] = deg_all[lo + perms[core][:nc_nodes]]
    Dg = []
    for g in range(g_groups):
        dmax = int(deg_sorted[:, g * P : (g + 1) * P].max())
        Dg.append(max(4, -(-dmax // NSPLIT) * NSPLIT))
    Dg = tuple(Dg)

    def padded_row(node):
        # row of a node in the (degree-sorted) gather table
        return (node // nc_nodes) * npad + inv_slot[node]

    in_maps = []
    for core in range(NCORES):
        lo, hi = core * nc_nodes, (core + 1) * nc_nodes
        sel = (dst >= lo) & (dst < hi)
        d_loc = inv_slot[dst[sel]]          # slot of dst within this core
        s_glb = src[sel]
        order = np.argsort(d_loc, kind="stable")
        d_sort = d_loc[order]
        s_sort = s_glb[order]
        starts = np.searchsorted(d_sort, np.arange(nc_nodes))
        rank = np.arange(len(d_sort)) - starts[d_sort]

        # dummy slot: a REAL node another core owns (msk=0 kills its term;
        # must not collide with pad-dst positions or distance could be 0)
        dummy = ((core + 1) % NCORES) * npad
        idx_cols = []
        msk_cols = []
        for g in range(g_groups):
            D = Dg[g]
            idx = np.full((P, D), dummy, np.int64)
            msk = np.zeros((P, D), np.float32)
            in_g = (d_sort >= g * P) & (d_sort < (g + 1) * P)
            rg = d_sort[in_g] - g * P
            rk = rank[in_g]
            idx[rg, rk] = padded_row(s_sort[in_g])
            msk[rg, rk] = np.sqrt(2.0, dtype=np.float32)
            idx_cols.append(_wrap_idx16(idx))
            msk_cols.append(msk)
        idx_w = np.concatenate(idx_cols, axis=1)
        msk_w = np.concatenate(msk_cols, axis=1)

        perm = perms[core]
        real = perm < nc_nodes

        posd = np.zeros((npad, 3), np.float32)
        posd[real] = pos[lo + perm[real]]
        # pad owners: any pos distinct from every gathered row
        posd[~real, 0] = 1e4 + np.arange(npad - nc_nodes, dtype=np.float32)

        h0loc = np.zeros((npad, c_ch), np.float32)
        h0loc[real] = h0[lo + perm[real], :, 0]

        def node_w(warr, layer):
            wloc = np.zeros((npad, c_ch * c_ch), np.float32)
            # pack transposed: flat (k, j) with w[j, k] at k*C + j
            wloc[real] = (
                warr[layer, 0, lo + perm[real]]
                .transpose(0, 2, 1)
                .reshape(-1, c_ch * c_ch)
            )
            return wloc.reshape(g_groups, P, c_ch * c_ch)

        predw = np.zeros((npad, 2), np.float32)
        predw[real] = pred_w[:, lo + perm[real]].T

        rep = lambda v: np.broadcast_to(
            np.asarray(v, np.float32).reshape(1, -1), (P, np.asarray(v).size)
        ).copy()

        def part_major(a):
            # [npad, K] -> [P, G*K] with group-major columns
            K = a.shape[1]
            return (
                a.reshape(g_groups, P, K).transpose(1, 0, 2).reshape(P, g_groups * K)
            )

        in_maps.append(
            {
                "idx_in": idx_w,
                "msk_in": msk_w,
                "posd_in": part_major(posd),
                "h0_in": part_major(h0loc),
                "cw0_in": node_w(cw, 0),
                "mw0_in": node_w(mw, 0),
                "cw1_in": node_w(cw, 1),
                "mw1_in": node_w(mw, 1),
                "abw0_in": rep(ab_w[0, 0].T.ravel()),
                "abw1_in": rep(ab_w[1, 0].T.ravel()),
                "wsw0_in": rep(ws_w[0, 0].ravel()),
                "wsw1_in": rep(ws_w[1, 0].ravel()),
                "cvec_in": rep(np.arange(1, c_ch + 1, dtype=np.float32) / 2.0),
                "predw_in": part_major(predw),
                "predb_in": (pred_b if core == 0 else np.zeros(2)).reshape(1, 2)
                .astype(np.float32),
                "ones_in": np.ones((P, 1), np.float32),
            }
        )
    meta = dict(Dg=Dg, C=c_ch, npad=npad)
    return in_maps, meta


_NC_CACHE = {}


def kernel(**inputs) -> np.ndarray:
    in_maps, meta = _prep_inputs(**inputs)
    key = (meta["Dg"], meta["C"], meta["npad"])
    if key not in _NC_CACHE:
        _NC_CACHE[key] = _build_nc(**meta)
    nc = _NC_CACHE[key]
    res = run_bass_kernel_spmd(nc, in_maps, core_ids=list(range(NCORES)))
    parts = [r["out_part"] for r in res.results]
    return np.sum(parts, axis=0).astype(np.float32)


# revision 8
# speedup vs baseline: 3.9020x; 1.0370x over previous
"""CartesianMACE rank-0 kernel for 8 Trainium2 NeuronCores (Bass/Tile).

Only the rank-0 (scalar) channel chain affects the model output: the final
readout is sum_j h[0][:, j, 0], and h[0]'s update depends only on A[0]
(which depends only on h[0] via the scalar atomic basis) and msgs[0]
(a polynomial in A[0]).  Ranks 1/2 are dead code for this output.

Per layer l:
  hm   = h0s @ ab_w[l,0].T                                  [N,C]
  A0   = segsum(remb[e,:] * hm[src_e,:], dst)               [N,C]
  msgs = (w1 + w2*A0 + w3*A0^2) * A0 (elementwise, per ch)  [N,C]
  h0s' = einsum('njk,nk->nj', cw[l,0], h0s)
       + einsum('njk,nk->nj', mw[l,0], msgs)
out[k] = sum_n (sum_j h0s_final[n,j]) * pred_w[k,n] + pred_b[k]

Sharding: nodes are dst-sharded across the 8 cores; within a core, nodes
are sorted by in-degree (descending) and grouped into G groups of 128
partitions with a per-group padded degree D_g (multiple of 4) — this cuts
padded edge slots ~1.7x vs a global max degree.  Edges are bucketed
host-side by dst into per-node degree slots.  Each layer's hm table (plus
pos for layer 0's fused distance computation) is AllGathered compactly
across the cores, widened into a 256-byte-row gather table, and edge rows
are fetched with dma_gather (InstDMAGatherAnt).  dma_gather descriptor
generation on the Pool engine runs on one Q7 core-pair per SWDGE queue, so
calls rotate across 4 queues (num_swdge_queues=4) for 4x desc-gen
throughput.  The [N,C,C] rank-0 cw/mw weights are node-sharded/streamed.
"""

import sys

import numpy as np

if "/opt/trn_rl_repo" not in sys.path:
    sys.path.insert(0, "/opt/trn_rl_repo")

import concourse.bacc as bacc
import concourse.mybir as mybir
import concourse.tile as tile
from concourse import library_config
from concourse.bass_utils import run_bass_kernel_spmd

NCORES = 8
P = 128
ROW = 64  # gather-table row width in fp32 elems (256B, dma_gather minimum)
NSPLIT = 4  # gather splits per group == SWDGE queue rotation width

F32 = mybir.dt.float32
I16 = mybir.dt.int16


def _build_nc(Dg, C, npad):
    """Build the SPMD Bass program.

    Dg: tuple of per-group padded max in-degrees (len G, each % 4 == 0);
    C: channels; npad: padded nodes per core (G*128).
    """
    Dg = list(Dg)
    G = len(Dg)
    nc = bacc.Bacc(
        "TRN2",
        target_bir_lowering=False,
        debug=False,
        num_devices=NCORES,
        num_swdge_queues=NSPLIT,
    )
    w0 = C + 4  # compact layer-0 row: hm | pos | pad
    ntab = NCORES * npad
    IW = sum(8 * d for d in Dg)   # int16 idx columns total
    MW = sum(Dg)                  # mask columns total
    RW = sum(d * C for d in Dg)   # remb columns total
    # per-group offsets
    off_i = np.cumsum([0] + [8 * d for d in Dg]).tolist()
    off_m = np.cumsum([0] + list(Dg)).tolist()
    off_r = np.cumsum([0] + [d * C for d in Dg]).tolist()

    qctr = [0]

    def next_q():
        q = qctr[0] % NSPLIT
        qctr[0] += 1
        return q

    # ---- I/O ----
    idx_in = nc.dram_tensor("idx_in", [P, IW], I16, kind="ExternalInput")
    msk_in = nc.dram_tensor("msk_in", [P, MW], F32, kind="ExternalInput")
    posd_in = nc.dram_tensor("posd_in", [P, G * 3], F32, kind="ExternalInput")
    h0_in = nc.dram_tensor("h0_in", [P, G * C], F32, kind="ExternalInput")
    cw0_in = nc.dram_tensor("cw0_in", [G, P, C * C], F32, kind="ExternalInput")
    mw0_in = nc.dram_tensor("mw0_in", [G, P, C * C], F32, kind="ExternalInput")
    cw1_in = nc.dram_tensor("cw1_in", [G, P, C * C], F32, kind="ExternalInput")
    mw1_in = nc.dram_tensor("mw1_in", [G, P, C * C], F32, kind="ExternalInput")
    abw0_in = nc.dram_tensor("abw0_in", [P, C * C], F32, kind="ExternalInput")
    abw1_in = nc.dram_tensor("abw1_in", [P, C * C], F32, kind="ExternalInput")
    wsw0_in = nc.dram_tensor("wsw0_in", [P, 3 * C], F32, kind="ExternalInput")
    wsw1_in = nc.dram_tensor("wsw1_in", [P, 3 * C], F32, kind="ExternalInput")
    cvec_in = nc.dram_tensor("cvec_in", [P, C], F32, kind="ExternalInput")
    predw_in = nc.dram_tensor("predw_in", [P, G * 2], F32, kind="ExternalInput")
    predb_in = nc.dram_tensor("predb_in", [1, 2], F32, kind="ExternalInput")
    ones_in = nc.dram_tensor("ones_in", [P, 1], F32, kind="ExternalInput")
    out_t = nc.dram_tensor("out_part", [1, 2], F32, kind="ExternalOutput")

    groups = [list(range(NCORES))]

    with tile.TileContext(nc) as tc:
        with (
            tc.tile_pool(name="const", bufs=1) as cpool,
            tc.tile_pool(name="resident", bufs=1) as rpool,
            tc.tile_pool(name="gath", bufs=4) as gpool,
            tc.tile_pool(name="work", bufs=2) as wpool,
            tc.tile_pool(name="small", bufs=3) as spool,
            tc.tile_pool(name="wstream", bufs=3) as wspool,
            tc.tile_pool(name="psum", bufs=1, space="PSUM") as ppool,
            tc.tile_pool(name="dram", bufs=1, space="DRAM") as dpool,
        ):
            nc.gpsimd.load_library(library_config.mlp)

            # ---- constants / resident data ----
            cvec_sb = cpool.tile([P, C], F32)
            abw0_sb = cpool.tile([P, C * C], F32)
            abw1_sb = cpool.tile([P, C * C], F32)
            wsw0_sb = cpool.tile([P, 3 * C], F32)
            wsw1_sb = cpool.tile([P, 3 * C], F32)
            ones_sb = cpool.tile([P, 1], F32)
            predb_sb = cpool.tile([1, 2], F32)
            nc.sync.dma_start(abw0_sb[:], abw0_in[:])

            idx_sb = rpool.tile([P, IW], I16)
            msk_sb = rpool.tile([P, MW], F32)
            posd_sb = rpool.tile([P, G * 3], F32)
            h0_sb = rpool.tile([P, G * C], F32)
            predw_sb = rpool.tile([P, G * 2], F32)
            remb_sb = rpool.tile([P, RW], F32)
            h1_sb = rpool.tile([P, G * C], F32)
            acc_sb = rpool.tile([P, 2], F32)
            nc.sync.dma_start(posd_sb[:], posd_in[:])
            nc.sync.dma_start(h0_sb[:], h0_in[:])
            nc.vector.memset(acc_sb[:], 0.0)

            # ---- DRAM buffers: compact AllGather bounce + wide gather table
            ag0_in = dpool.tile([npad, w0], F32)
            ag0_out = dpool.tile([ntab, w0], F32)
            ag1_in = dpool.tile([npad, C], F32)
            ag1_out = dpool.tile([ntab, C], F32)
            wtab = dpool.tile([ntab, ROW], F32)

            def hm_group(g, h_src_ap, abw_sb, agin):
                """hm[:, j] = sum_k h[:, k] * abw[j, k]; DMA into agin rows."""
                t = wpool.tile([P, C * C], F32, name=f"hmT_{g}", tag="hmT")
                nc.vector.tensor_mul(
                    out=t[:].rearrange("p (k j) -> p k j", j=C),
                    in0=abw_sb[:].rearrange("p (k j) -> p k j", j=C),
                    in1=h_src_ap.to_broadcast([P, C, C]),
                )
                hm = spool.tile([P, C], F32, name=f"hm_{g}", tag="hm")
                nc.vector.reduce_sum(
                    out=hm[:],
                    in_=t[:].rearrange("p (k j) -> p j k", j=C),
                    axis=mybir.AxisListType.X,
                )
                dst = agin[:].rearrange("(g p) w -> p g w", p=P)[:, g, 0:C]
                nc.sync.dma_start(dst, hm[:])

            # ---- layer 0 hm (from input h0) + pos columns ----
            for g in range(G):
                hm_group(g, h0_sb[:, g * C : (g + 1) * C], abw0_sb, ag0_in)
                pdst = ag0_in[:].rearrange("(g p) w -> p g w", p=P)[:, g, C : C + 3]
                nc.sync.dma_start(pdst, posd_sb[:, g * 3 : (g + 1) * 3])

            nc.gpsimd.collective_compute(
                "AllGather",
                mybir.AluOpType.bypass,
                replica_groups=groups,
                ins=[ag0_in[:].opt()],
                outs=[ag0_out[:].opt()],
            )
            nc.sync.dma_start(idx_sb[:], idx_in[:])
            nc.sync.dma_start(msk_sb[:], msk_in[:])
            nc.sync.dma_start(cvec_sb[:], cvec_in[:])
            nc.sync.dma_start(abw1_sb[:], abw1_in[:])
            nc.sync.dma_start(wsw0_sb[:], wsw0_in[:])
            nc.sync.dma_start(wsw1_sb[:], wsw1_in[:])
            nc.sync.dma_start(ones_sb[:], ones_in[:])
            nc.sync.dma_start(predb_sb[:], predb_in[:])
            nc.sync.dma_start(predw_sb[:], predw_in[:])
            # widen compact rows into the 256B-row gather table; split into
            # parallel chunks — one instruction would drain ntab tiny
            # segments through a single HWDGE queue (~70us serial stall)
            wchunk = ntab // 8
            for w in range(8):
                nc.sync.dma_start(
                    wtab[w * wchunk : (w + 1) * wchunk, 0 : C + 3],
                    ag0_out[w * wchunk : (w + 1) * wchunk, 0 : C + 3],
                )

            Dmax = max(Dg)

            def edge_layer(g, remb_ready):
                """Gather rows, (layer 0 only) build remb, reduce to A0."""
                D = Dg[g]
                gt = gpool.tile([P, Dmax * ROW], F32, name=f"gt_{g}", tag="gt")
                dq = D // NSPLIT
                for s in range(NSPLIT):
                    nc.gpsimd.dma_gather(
                        out_ap=gt[:].rearrange("p (d e) -> p d e", e=ROW)[
                            :, s * dq : (s + 1) * dq, :
                        ],
                        in_ap=wtab[:],
                        idxs_ap=idx_sb[
                            :,
                            off_i[g] + s * 8 * dq : off_i[g] + (s + 1) * 8 * dq,
                        ],
                        num_idxs=P * dq,
                        num_idxs_reg=P * dq,
                        elem_size=ROW,
                        single_packet=False,
                        queue_num=next_q(),
                    )
                gv = gt[:].rearrange("p (d e) -> p d e", e=ROW)[:, 0:D, :]
                he = gt[:].rearrange("p (d e) -> p d e", e=ROW)[:, 0:D, 0:C]
                remb_g = remb_sb[:, off_r[g] : off_r[g] + D * C]
                if not remb_ready:
                    # distances from gathered src pos vs resident dst pos
                    rel_t = spool.tile([P, Dmax * 3], F32, name=f"rel_{g}", tag="rel")
                    rel = rel_t[:, 0 : D * 3]
                    posd_g = posd_sb[:, g * 3 : (g + 1) * 3]
                    nc.vector.tensor_sub(
                        out=rel.rearrange("p (d x) -> p d x", x=3),
                        in0=gv[:, :, C : C + 3],
                        in1=posd_g.to_broadcast([P, 3, D]).rearrange(
                            "p x d -> p d x"
                        ),
                    )
                    sq_t = spool.tile([P, Dmax * 3], F32, name=f"sq_{g}", tag="sq")
                    sq = sq_t[:, 0 : D * 3]
                    nc.vector.tensor_mul(out=sq, in0=rel, in1=rel)
                    dd_t = spool.tile([P, Dmax], F32, name=f"dd_{g}", tag="dd")
                    dd = dd_t[:, 0:D]
                    nc.vector.reduce_sum(
                        out=dd,
                        in_=sq.rearrange("p (d x) -> p d x", x=3),
                        axis=mybir.AxisListType.X,
                    )
                    dv_t = spool.tile([P, Dmax], F32, name=f"dv_{g}", tag="dv")
                    dv = dv_t[:, 0:D]
                    nc.scalar.sqrt(dv, dd)
                    inv_t = spool.tile([P, Dmax], F32, name=f"inv_{g}", tag="inv")
                    inv = inv_t[:, 0:D]
                    nc.vector.reciprocal(inv, dv)
                    wv_t = spool.tile([P, Dmax], F32, name=f"wv_{g}", tag="wv")
                    wv = wv_t[:, 0:D]
                    nc.vector.tensor_mul(
                        out=wv, in0=inv, in1=msk_sb[:, off_m[g] : off_m[g] + D]
                    )
                    # sinin = x2 = (c+1)*d/2 (cvec holds (c+1)/2); k = round(x2)
                    # via the fp32 magic trick; sin(pi*(c+1)*d) = sin(2pi*(x2-k))
                    sin_t = wpool.tile([P, Dmax * C], F32, name=f"sinin_{g}", tag="sin")
                    sinin = sin_t[:, 0 : D * C]
                    nc.vector.tensor_mul(
                        out=sinin.rearrange("p (d c) -> p d c", c=C),
                        in0=dv.to_broadcast([P, D, C]),
                        in1=cvec_sb[:].to_broadcast([P, C, D]).rearrange(
                            "p c d -> p d c"
                        ),
                    )
                    MAGIC = 12582912.0  # 1.5 * 2**23
                    kr_t = wpool.tile([P, Dmax * C], F32, name=f"kr_{g}", tag="kr")
                    kr = kr_t[:, 0 : D * C]
                    nc.vector.tensor_scalar(
                        out=kr,
                        in0=sinin,
                        scalar1=MAGIC,
                        scalar2=MAGIC,
                        op0=mybir.AluOpType.add,
                        op1=mybir.AluOpType.subtract,
                    )
                    nc.vector.tensor_sub(out=sinin, in0=sinin, in1=kr)
                    sino_t = wpool.tile([P, Dmax * C], F32, name=f"sino_{g}", tag="sino")
                    sino = sino_t[:, 0 : D * C]
                    nc.scalar.activation(
                        sino,
                        sinin,
                        mybir.ActivationFunctionType.Sin,
                        scale=float(2.0 * np.pi),
                    )
                    nc.vector.tensor_mul(
                        out=remb_g.rearrange("p (d c) -> p d c", c=C),
                        in0=sino.rearrange("p (d c) -> p d c", c=C),
                        in1=wv.to_broadcast([P, D, C]),
                    )
                # edge products, in place over the gathered hm columns
                nc.vector.tensor_mul(
                    out=he,
                    in0=remb_g.rearrange("p (d c) -> p d c", c=C),
                    in1=he,
                )
                a0 = spool.tile([P, C], F32, name=f"a0_{g}", tag="a0")
                nc.vector.reduce_sum(
                    out=a0[:],
                    in_=gt[:].rearrange("p (d e) -> p e d", e=ROW)[:, 0:C, 0:D],
                    axis=mybir.AxisListType.X,
                )
                return a0

            def node_update(g, a0, wsw_sb, cw_in_t, mw_in_t, h_src_ap, h_dst_ap):
                """msgs = (w1 + w2*s + w3*s^2)*A0; h' = cw@h + mw@msgs."""
                s2 = spool.tile([P, C], F32, name=f"s2_{g}", tag="s2")
                nc.scalar.square(s2[:], a0[:])
                s3 = spool.tile([P, C], F32, name=f"s3_{g}", tag="s3")
                nc.vector.tensor_mul(out=s3[:], in0=s2[:], in1=a0[:])
                m1 = spool.tile([P, C], F32, name=f"m1_{g}", tag="m1")
                nc.vector.tensor_mul(out=m1[:], in0=a0[:], in1=wsw_sb[:, 0:C])
                m2 = spool.tile([P, C], F32, name=f"m2_{g}", tag="m2")
                nc.vector.tensor_mul(out=m2[:], in0=s2[:], in1=wsw_sb[:, C : 2 * C])
                m3 = spool.tile([P, C], F32, name=f"m3_{g}", tag="m3")
                nc.vector.tensor_mul(
                    out=m3[:], in0=s3[:], in1=wsw_sb[:, 2 * C : 3 * C]
                )
                ms = spool.tile([P, C], F32, name=f"ms_{g}", tag="ms")
                nc.vector.tensor_add(out=ms[:], in0=m1[:], in1=m2[:])
                msgs = spool.tile([P, C], F32, name=f"msgs_{g}", tag="msgs")
                nc.vector.tensor_add(out=msgs[:], in0=ms[:], in1=m3[:])

                cw_t = wspool.tile([P, C * C], F32, name=f"cw_{g}", tag="cw")
                mw_t = wspool.tile([P, C * C], F32, name=f"mw_{g}", tag="mw")
                nc.sync.dma_start(cw_t[:], cw_in_t[g])
                nc.sync.dma_start(mw_t[:], mw_in_t[g])
                t1 = wpool.tile([P, C * C], F32, name=f"t1_{g}", tag="t1")
                nc.vector.tensor_mul(
                    out=t1[:].rearrange("p (k j) -> p k j", j=C),
                    in0=cw_t[:].rearrange("p (k j) -> p k j", j=C),
                    in1=h_src_ap.to_broadcast([P, C, C]),
                )
                t2 = wpool.tile([P, C * C], F32, name=f"t2_{g}", tag="t2")
                nc.vector.tensor_mul(
                    out=t2[:].rearrange("p (k j) -> p k j", j=C),
                    in0=mw_t[:].rearrange("p (k j) -> p k j", j=C),
                    in1=msgs[:].to_broadcast([P, C, C]),
                )
                ts = wpool.tile([P, C * C], F32, name=f"ts_{g}", tag="ts")
                nc.vector.tensor_add(out=ts[:], in0=t1[:], in1=t2[:])
                nc.vector.reduce_sum(
                    out=h_dst_ap,
                    in_=ts[:].rearrange("p (k j) -> p j k", j=C),
                    axis=mybir.AxisListType.X,
                )

            # ---- layer 0 ----
            for g in range(G):
                a0 = edge_layer(g, remb_ready=False)
                h1_g = h1_sb[:, g * C : (g + 1) * C]
                node_update(
                    g, a0, wsw0_sb, cw0_in, mw0_in,
                    h0_sb[:, g * C : (g + 1) * C], h1_g,
                )
                hm_group(g, h1_g, abw1_sb, ag1_in)

            nc.gpsimd.collective_compute(
                "AllGather",
                mybir.AluOpType.bypass,
                replica_groups=groups,
                ins=[ag1_in[:].opt()],
                outs=[ag1_out[:].opt()],
            )
            for w in range(8):
                nc.sync.dma_start(
                    wtab[w * wchunk : (w + 1) * wchunk, 0:C],
                    ag1_out[w * wchunk : (w + 1) * wchunk, 0:C],
                )

            # ---- layer 1 + head ----
            for g in range(G):
                a1 = edge_layer(g, remb_ready=True)
                hf = spool.tile([P, C], F32, name=f"hf_{g}", tag="hf")
                node_update(
                    g, a1, wsw1_sb, cw1_in, mw1_in,
                    h1_sb[:, g * C : (g + 1) * C], hf[:],
                )
                rs = spool.tile([P, 1], F32, name=f"rs_{g}", tag="rs")
                nc.vector.reduce_sum(out=rs[:], in_=hf[:], axis=mybir.AxisListType.X)
                ctr = spool.tile([P, 2], F32, name=f"ctr_{g}", tag="ctr")
                nc.vector.tensor_mul(
                    out=ctr[:],
                    in0=predw_sb[:, g * 2 : (g + 1) * 2],
                    in1=rs[:].to_broadcast([P, 2]),
                )
                nc.vector.tensor_add(out=acc_sb[:], in0=acc_sb[:], in1=ctr[:])

            # cross-partition sum via PE, add bias, write out
            fin_ps = ppool.tile([1, 2], F32)
            nc.tensor.matmul(
                out=fin_ps[:], lhsT=ones_sb[:], rhs=acc_sb[:], start=True, stop=True
            )
            res = spool.tile([1, 2], F32)
            nc.vector.tensor_add(out=res[:], in0=fin_ps[:], in1=predb_sb[:])
            nc.sync.dma_start(out_t[:], res[:])

    nc.compile()
    return nc


def _wrap_idx16(v):
    """[128, D] int array -> dma_gather idx tile [128, 8*D] int16.

    Gathered index j = d*128 + p must sit at [j % 16, j // 16] in a
    16-partition wrap, replicated 8x down the partitions."""
    p128, d_pad = v.shape
    blk = v.T.reshape(d_pad, 8, 16).transpose(2, 0, 1).reshape(16, 8 * d_pad)
    return np.tile(blk, (8, 1)).astype(np.int16)


def _prep_inputs(pos, h0, ab_w, ws_w, cw, mw, pred_w, pred_b, edge_index):
    """Host-side sharding: degree-sort nodes per core, bucket edges by dst
    into per-group padded degree slots, slice per-node weights.  Data
    movement / index arithmetic only — all model arithmetic runs on
    device."""
    pos = np.asarray(pos, np.float32)
    h0 = np.asarray(h0, np.float32)
    ab_w = np.asarray(ab_w, np.float32)
    ws_w = np.asarray(ws_w, np.float32)
    cw = np.asarray(cw, np.float32)
    mw = np.asarray(mw, np.float32)
    pred_w = np.asarray(pred_w, np.float32)
    pred_b = np.asarray(pred_b, np.float32)
    ei = np.asarray(edge_index)
    src = ei[0].astype(np.int64)
    dst = ei[1].astype(np.int64)

    n_nodes = pos.shape[0]
    c_ch = h0.shape[1]
    assert n_nodes % NCORES == 0
    nc_nodes = n_nodes // NCORES  # real nodes per core
    g_groups = -(-nc_nodes // P)
    npad = g_groups * P
    assert NCORES * npad <= 32767, "int16 gather indices"

    deg_all = np.bincount(dst, minlength=n_nodes)

    # per-core degree-descending node permutation (padding nodes at end)
    perms = []     # perms[core][slot] = local real node id (or >=nc_nodes pad)
    inv_slot = np.zeros(n_nodes, np.int64)  # global node -> slot within core
    for core in range(NCORES):
        lo = core * nc_nodes
        order = np.argsort(-deg_all[lo : lo + nc_nodes], kind="stable")
        perm = np.concatenate([order, np.arange(nc_nodes, npad)])
        perms.append(perm)
        inv_slot[lo + order] = np.arange(nc_nodes)

    # per-group padded degree (shared across cores so one program serves all)
    deg_sorted = np.zeros((NCORES, npad), np.int64)
    for core in range(NCORES):
        lo = core * nc_nodes
        deg_sorted[core, :nc_nodes] = deg_all[lo + perms[core][:nc_nodes]]
    Dg = []
    for g in range(g_groups):
        dmax = int(deg_sorted[:, g * P : (g + 1) * P].max())
        Dg.append(max(4, -(-dmax // NSPLIT) * NSPLIT))
    Dg = tuple(Dg)

    def padded_row(node):
        # row of a node in the (degree-sorted) gather table
        return (node // nc_nodes) * npad + inv_slot[node]

    in_maps = []
    for core in range(NCORES):
        lo, hi = core * nc_nodes, (core + 1) * nc_nodes
        sel = (dst >= lo) & (dst < hi)
        d_loc = inv_slot[dst[sel]]          # slot of dst within this core
        s_glb = src[sel]
        order = np.argsort(d_loc, kind="stable")
        d_sort = d_loc[order]
        s_sort = s_glb[order]
        starts = np.searchsorted(d_sort, np.arange(nc_nodes))
        rank = np.arange(len(d_sort)) - starts[d_sort]

        # dummy slot: a REAL node another core owns (msk=0 kills its term;
        # must not collide with pad-dst positions or distance could be 0)
        dummy = ((core + 1) % NCORES) * npad
        idx_cols = []
        msk_cols = []
        for g in range(g_groups):
            D = Dg[g]
            idx = np.full((P, D), dummy, np.int64)
            msk = np.zeros((P, D), np.float32)
            in_g = (d_sort >= g * P) & (d_sort < (g + 1) * P)
            rg = d_sort[in_g] - g * P
            rk = rank[in_g]
            idx[rg, rk] = padded_row(s_sort[in_g])
            msk[rg, rk] = np.sqrt(2.0, dtype=np.float32)
            idx_cols.append(_wrap_idx16(idx))
            msk_cols.append(msk)
        idx_w = np.concatenate(idx_cols, axis=1)
        msk_w = np.concatenate(msk_cols, axis=1)

        perm = perms[core]
        real = perm < nc_nodes

        posd = np.zeros((npad, 3), np.float32)
        posd[real] = pos[lo + perm[real]]
        # pad owners: any pos distinct from every gathered row
        posd[~real, 0] = 1e4 + np.arange(npad - nc_nodes, dtype=np.float32)

        h0loc = np.zeros((npad, c_ch), np.float32)
        h0loc[real] = h0[lo + perm[real], :, 0]

        def node_w(warr, layer):
            wloc = np.zeros((npad, c_ch * c_ch), np.float32)
            # pack transposed: flat (k, j) with w[j, k] at k*C + j
            wloc[real] = (
                warr[layer, 0, lo + perm[real]]
                .transpose(0, 2, 1)
                .reshape(-1, c_ch * c_ch)
            )
            return wloc.reshape(g_groups, P, c_ch * c_ch)

        predw = np.zeros((npad, 2), np.float32)
        predw[real] = pred_w[:, lo + perm[real]].T

        rep = lambda v: np.broadcast_to(
            np.asarray(v, np.float32).reshape(1, -1), (P, np.asarray(v).size)
        ).copy()

        def part_major(a):
            # [npad, K] -> [P, G*K] with group-major columns
            K = a.shape[1]
            return (
                a.reshape(g_groups, P, K).transpose(1, 0, 2).reshape(P, g_groups * K)
            )

        in_maps.append(
            {
                "idx_in": idx_w,
                "msk_in": msk_w,
                "posd_in": part_major(posd),
                "h0_in": part_major(h0loc),
                "cw0_in": node_w(cw, 0),
                "mw0_in": node_w(mw, 0),
                "cw1_in": node_w(cw, 1),
                "mw1_in": node_w(mw, 1),
                "abw0_in": rep(ab_w[0, 0].T.ravel()),
                "abw1_in": rep(ab_w[1, 0].T.ravel()),
                "wsw0_in": rep(ws_w[0, 0].ravel()),
                "wsw1_in": rep(ws_w[1, 0].ravel()),
                "cvec_in": rep(np.arange(1, c_ch + 1, dtype=np.float32) / 2.0),
                "predw_in": part_major(predw),
                "predb_in": (pred_b if core == 0 else np.zeros(2)).reshape(1, 2)
                .astype(np.float32),
                "ones_in": np.ones((P, 1), np.float32),
            }
        )
    meta = dict(Dg=Dg, C=c_ch, npad=npad)
    return in_maps, meta


_NC_CACHE = {}


def kernel(**inputs) -> np.ndarray:
    in_maps, meta = _prep_inputs(**inputs)
    key = (meta["Dg"], meta["C"], meta["npad"])
    if key not in _NC_CACHE:
        _NC_CACHE[key] = _build_nc(**meta)
    nc = _NC_CACHE[key]
    res = run_bass_kernel_spmd(nc, in_maps, core_ids=list(range(NCORES)))
    parts = [r["out_part"] for r in res.results]
    return np.sum(parts, axis=0).astype(np.float32)
